# revision 15
# baseline (speedup 1.0000x reference)
"""Trainium2 Bass kernel for the LoRA-BC block (nn_LoRABCBlock).

Computation (per reference):
    base = x @ w_base.T
    h = layernorm(x) * gamma + beta
    qkv = h @ w_qkv.T ; attention (2 heads, head_dim 32) over full sequence
    attn_out = attn_output @ w_attn_out.T
    delta = ((h + attn_out) @ lora_down) @ lora_up
    out = base + (1/8) * delta

Sharding: data-parallel over (batch, seq-half) -> 8 cores. Weights
replicated; each core computes k/v over its batch's full 2048 rows.

v3 design notes:
  - PE runs only real matmuls + a few small weight transposes. Big
    transposes (x, z=normalized x, w_base) go through the DMA XBAR
    (dma_start_transpose, bf16). All XBAR transposes are issued on ONE
    HWDGE ring (scalar) so they serialize mutually -- concurrent
    transposes from two rings corrupt each other's xbar state. x loads
    and output stores ride the sync ring so transposes never block them.
  - Attention computes transposed scores scoresT[sk, sq] (lhsT=kT,
    rhs=qT), exp'd in [128, 2048] batches (ACT per-op overhead dominates
    small tiles), and attn@v consumes them directly with a ones column
    appended to V producing the softmax denominator in the same psum.
  - gamma is folded into the qkv weights; beta enters qkv as a rank-1
    matmul, and h(+attn_out) is reconstructed as zh*gamma + (p4 + beta)
    via scalar_tensor_tensor, so no separate gamma/beta pass exists.
  - Output is stored bf16 and widened on the host.
"""

import sys

sys.path.insert(0, "/opt/trn_rl_repo")

from contextlib import ExitStack

import numpy as np

import concourse.bass as bass
import concourse.tile as tile
from concourse import bacc, mybir
from concourse.bass_utils import run_bass_kernel_spmd
from concourse.masks import make_identity

F32 = mybir.dt.float32
BF16 = mybir.dt.bfloat16
AF = mybir.ActivationFunctionType
ALU = mybir.AluOpType

E = 1024          # embed dim
DM = 1024         # d_model
R = 8             # lora rank
SCALING = 1.0 / R
DA = 64           # attn dim
NH = 2            # heads
HD = DA // NH     # head dim = 32
SOWN = 1024       # rows owned per core
SFULL = 2048      # rows per batch element
NC = 8            # cores
P = 128
KT = E // P       # 8 k-tiles
MT = SOWN // P    # 8 own m-tiles
ST = SFULL // P   # 16 sequence tiles
ATT_SCALE = float(HD) ** -0.5


def build_kernel(dbg=False):
    nc = bacc.Bacc("TRN2", target_bir_lowering=False, debug=False, num_devices=NC)

    x_own = nc.dram_tensor("x_own", [SOWN, E], F32, kind="ExternalInput").ap()
    x_oth = nc.dram_tensor("x_oth", [SOWN, E], F32, kind="ExternalInput").ap()
    w_base = nc.dram_tensor("w_base", [DM, E], F32, kind="ExternalInput").ap()
    ln_g = nc.dram_tensor("ln_g", [E], F32, kind="ExternalInput").ap()
    ln_b = nc.dram_tensor("ln_b", [E], F32, kind="ExternalInput").ap()
    ld = nc.dram_tensor("ld", [E, R], F32, kind="ExternalInput").ap()
    lu = nc.dram_tensor("lu", [R, DM], F32, kind="ExternalInput").ap()
    w_qkv = nc.dram_tensor("w_qkv", [3 * DA, E], F32, kind="ExternalInput").ap()
    w_ao = nc.dram_tensor("w_ao", [E, DA], F32, kind="ExternalInput").ap()
    out_d = nc.dram_tensor("out", [SOWN, DM], BF16, kind="ExternalOutput").ap()
    dbg_d = {}
    if dbg:
        for nm, shp in [("zhT", [P, KT, SFULL]),
                        ("WbT", [P, KT, DM]), ("qT", [DA, SOWN]),
                        ("kTt", [DA, SFULL]), ("vT", [DA, SFULL]),
                        ("vaug0", [P, ST, 48]), ("vaug1", [P, ST, 48]),
                        ("aoT", [DA, SOWN]), ("hwaT", [P, KT, SOWN]),
                        ("tT", [R, SOWN])]:
            dbg_d[nm] = nc.dram_tensor("dbg_" + nm, shp, BF16,
                                       kind="ExternalOutput").ap()

    with tile.TileContext(nc) as tc, ExitStack() as ctx:
        persist = ctx.enter_context(tc.tile_pool(name="persist", bufs=1))
        ld_pool = ctx.enter_context(tc.tile_pool(name="loads", bufs=3))
        wb_pool = ctx.enter_context(tc.tile_pool(name="wbload", bufs=2))
        zh_pool = ctx.enter_context(tc.tile_pool(name="zh", bufs=3))
        st_pool = ctx.enter_context(tc.tile_pool(name="stats", bufs=4))
        ex_pool = ctx.enter_context(tc.tile_pool(name="expb", bufs=2))
        o_pool = ctx.enter_context(tc.tile_pool(name="outs", bufs=3))
        ps = ctx.enter_context(tc.tile_pool(name="ps", bufs=1, space="PSUM"))

        _psn = [0]

        def ps_tile(shape, dtype, tag, bufs):
            _psn[0] += 1
            return ps.tile(shape, dtype, tag=tag, bufs=bufs,
                           name=f"ps_{tag}_{_psn[0]}")

        # ---------------- constants ----------------
        ident = persist.tile([P, P], BF16, tag="ident")
        make_identity(nc, ident)
        eps_t = persist.tile([P, 1], F32, tag="eps")
        nc.vector.memset(eps_t, 1e-5)
        ones512 = persist.tile([1, 512], BF16, tag="ones512")
        nc.vector.memset(ones512, 1.0)
        # gamma/beta arranged [p, kt] (e = kt*128 + p)
        gT = persist.tile([P, KT], F32, tag="gT")
        bT = persist.tile([P, KT], F32, tag="bT")
        nc.scalar.dma_start(out=gT, in_=ln_g.rearrange("(kt p) -> p kt", p=P))
        nc.scalar.dma_start(out=bT, in_=ln_b.rearrange("(kt p) -> p kt", p=P))
        bTh = persist.tile([P, KT], BF16, tag="bTh")
        nc.vector.tensor_copy(out=bTh, in_=bT)
        # beta as a bf16 row [1, E] (for the rank-1 +beta in attn residual)
        bRow_f = ld_pool.tile([1, E], F32, tag="bRow_f", bufs=1)
        nc.scalar.dma_start(out=bRow_f, in_=ln_b.rearrange("(o e) -> o e", o=1))
        bRow = persist.tile([1, E], BF16, tag="bRow")
        nc.vector.tensor_copy(out=bRow, in_=bRow_f)

        # ---------------- persistent activations ----------------
        zhT = persist.tile([P, KT, SFULL], BF16, tag="zhT")   # [p_e, k, s] raw z
        qT = persist.tile([DA, SOWN], BF16, tag="qT")
        kTt = persist.tile([DA, SFULL], BF16, tag="kTt")
        vT = persist.tile([DA, SFULL], BF16, tag="vT")
        v_aug = [persist.tile([P, ST, 48], BF16, tag=f"vaug{h}", name=f"vaug{h}")
                 for h in range(NH)]
        aoT = persist.tile([DA, SOWN], BF16, tag="aoT")
        hwaT = persist.tile([P, KT, SOWN], BF16, tag="hwaT")
        tT = persist.tile([R, SOWN], BF16, tag="tT")

        # ---------------- weights (scalar-ring DMAs) ----------------
        # w_qkv -> wqkvT[k] [128e, 192a] via PE transposes; gamma folded in
        wqkvT = [persist.tile([P, 3 * DA], BF16, tag=f"wqkvT{k}", name=f"wqkvT{k}")
                 for k in range(KT)]
        wq0f = ld_pool.tile([P, E], F32, tag="wload", bufs=1)
        nc.scalar.dma_start(out=wq0f, in_=w_qkv[0:P, :])
        wq0h = persist.tile([P, E], BF16, tag="wq0h")
        nc.vector.tensor_copy(out=wq0h, in_=wq0f)
        wq1f = ld_pool.tile([64, E], F32, tag="wload1", bufs=1)
        nc.scalar.dma_start(out=wq1f, in_=w_qkv[P:3 * DA, :])
        wq1h = persist.tile([64, E], BF16, tag="wq1h")
        nc.vector.tensor_copy(out=wq1h, in_=wq1f)
        for k in range(KT):
            tp = ps_tile([P, P], BF16, "mm2", 2)
            nc.tensor.transpose(tp, wq0h[:, k * P:(k + 1) * P], ident)
            nc.vector.tensor_copy(out=wqkvT[k][:, 0:P], in_=tp)
            tp2 = ps_tile([P, 64], BF16, "mm2", 2)
            nc.tensor.transpose(tp2, wq1h[:, k * P:(k + 1) * P], ident[0:64, 0:64])
            nc.vector.tensor_copy(out=wqkvT[k][:, P:3 * DA], in_=tp2)

        # beta @ w_qkv.T  (before gamma folding), then fold gamma into wqkvT
        pbw = ps_tile([1, 3 * DA], F32, "mm2", 2)
        for k in range(KT):
            nc.tensor.matmul(pbw, bTh[:, k:k + 1], wqkvT[k],
                             start=(k == 0), stop=(k == KT - 1))
        bW = persist.tile([1, 3 * DA], BF16, tag="bW")
        nc.vector.tensor_copy(out=bW, in_=pbw)
        for k in range(KT):
            nc.vector.tensor_scalar_mul(out=wqkvT[k], in0=wqkvT[k],
                                        scalar1=gT[:, k:k + 1])

        # w_attn_out -> waoT [64d, 1024n] via PE transposes
        waoT = persist.tile([DA, E], BF16, tag="waoT")
        for ntile in range(KT):
            wf = ld_pool.tile([P, DA], F32, tag="waoload", bufs=2)
            nc.scalar.dma_start(out=wf, in_=w_ao[ntile * P:(ntile + 1) * P, :])
            wh = ld_pool.tile([P, DA], BF16, tag="waocast", bufs=2)
            nc.vector.tensor_copy(out=wh, in_=wf)
            tp = ps_tile([DA, P], BF16, "mm2", 2)
            nc.tensor.transpose(tp, wh, ident)
            nc.vector.tensor_copy(out=waoT[:, ntile * P:(ntile + 1) * P], in_=tp)

        # lora_down [E, R] -> [p, kt, r] bf16
        ld_f = ld_pool.tile([P, KT, R], F32, tag="ldload", bufs=1)
        nc.scalar.dma_start(out=ld_f, in_=ld.rearrange("(kt p) r -> p kt r", p=P))
        ld_sb = persist.tile([P, KT, R], BF16, tag="ld_sb")
        nc.vector.tensor_copy(out=ld_sb, in_=ld_f)

        # lora_up [R, DM] bf16, pre-scaled by SCALING
        lu_f = ld_pool.tile([R, DM], F32, tag="luload", bufs=1)
        nc.scalar.dma_start(out=lu_f, in_=lu)
        lu_sb = persist.tile([R, DM], BF16, tag="lu_sb")
        nc.scalar.mul(lu_sb, lu_f, SCALING)

        # w_base loads + casts (transposes are emitted LAST on the ring)
        wbh_t = []
        for ntile in range(KT):
            wbf = wb_pool.tile([P, E], F32, tag="wbf")
            nc.scalar.dma_start(out=wbf, in_=w_base[ntile * P:(ntile + 1) * P, :])
            wbh = wb_pool.tile([P, E], BF16, tag="wbh", bufs=8)
            nc.vector.tensor_copy(out=wbh, in_=wbf)
            wbh_t.append(wbh)

        stdAll = persist.tile([P, MT], F32, tag="stdAll")   # std per own tile
        muRow = persist.tile([1, SOWN], BF16, tag="muRow")  # mu as row (own)
        stdRow = persist.tile([1, SOWN], F32, tag="stdRow")  # std as row (own)

        # ---------------- phase 1: x load + layernorm -----------------
        def do_st(st):
            own = st < MT
            src = x_own if own else x_oth
            row0 = st * P if own else (st - MT) * P
            xf = ld_pool.tile([P, E], F32, tag="xin")
            nc.sync.dma_start(out=xf, in_=src[row0:row0 + P, :])

            stats = st_pool.tile([P, 2, 6], F32, tag="bnstats")
            xr = xf.rearrange("p (n f) -> p n f", f=512)
            for sg in range(2):
                nc.vector.bn_stats(out=stats[:, sg, :], in_=xr[:, sg, :])
            mv = st_pool.tile([P, 2], F32, tag="mv")
            nc.vector.bn_aggr(out=mv, in_=stats)
            rstd = st_pool.tile([P, 1], F32, tag="rstd")
            nc.scalar.activation(out=rstd, in_=mv[:, 1:2], func=AF.Sqrt, bias=eps_t)
            nc.vector.reciprocal(out=rstd, in_=rstd)
            nmr = st_pool.tile([P, 1], F32, tag="nmr")
            nc.vector.tensor_scalar(out=nmr, in0=mv[:, 0:1], scalar1=rstd,
                                    scalar2=-1.0, op0=ALU.mult, op1=ALU.mult)
            # z = (x - mu) * rstd   (bf16)
            zh = zh_pool.tile([P, E], BF16, tag="zh")
            nc.vector.tensor_scalar(out=zh, in0=xf, scalar1=rstd, scalar2=nmr,
                                    op0=ALU.mult, op1=ALU.add)
            if own:
                # std = 1/rstd and muRow (= mu, via -nmr/rstd... just -nmr*std)
                nc.vector.reciprocal(out=stdAll[:, st:st + 1], in_=rstd)
                # rows: transpose rstd and nmr to [1, 128] rows (partition 0)
                rn = st_pool.tile([P, 2], BF16, tag="rn")
                nc.vector.tensor_copy(out=rn[:, 0:1], in_=rstd)
                nc.vector.tensor_copy(out=rn[:, 1:2], in_=nmr)
                pr0 = ps_tile([1, P], F32, "mm2", 2)
                nc.tensor.matmul(pr0, rn[:, 0:1], ident, start=True, stop=True)
                pr1 = ps_tile([1, P], F32, "mm2", 2)
                nc.tensor.matmul(pr1, rn[:, 1:2], ident, start=True, stop=True)
                nc.vector.reciprocal(out=stdRow[:, st * P:(st + 1) * P],
                                     in_=pr0)
                # mu = -nmr * std  (bf16 row)
                nc.vector.scalar_tensor_tensor(
                    out=muRow[:, st * P:(st + 1) * P], in0=pr1,
                    scalar=-1.0, in1=stdRow[:, st * P:(st + 1) * P],
                    op0=ALU.mult, op1=ALU.mult)
            # transpose z via PE (regular matmuls against identity)
            for k in range(KT):
                tpz = ps_tile([P, P], F32, "tpz", 2)
                nc.tensor.matmul(tpz, zh[:, k * P:(k + 1) * P], ident,
                                 start=True, stop=True)
                eng = nc.vector if (k % 2 == 0) else nc.scalar
                if eng is nc.scalar:
                    nc.scalar.copy(out=zhT[:, k, st * P:(st + 1) * P], in_=tpz)
                else:
                    nc.vector.tensor_copy(out=zhT[:, k, st * P:(st + 1) * P],
                                          in_=tpz)

        def qkv_qk_own(grp):
            pq = ps_tile([P, 512], F32, "big", 2)
            for k in range(KT):
                nc.tensor.matmul(pq, wqkvT[k][:, 0:P],
                                 zhT[:, k, grp * 512:(grp + 1) * 512],
                                 start=(k == 0), stop=False)
            nc.tensor.matmul(pq, bW[:, 0:P], ones512, start=False, stop=True)
            nc.vector.tensor_copy(out=qT[:, grp * 512:(grp + 1) * 512],
                                  in_=pq[0:DA, :])
            nc.vector.tensor_copy(out=kTt[:, grp * 512:(grp + 1) * 512],
                                  in_=pq[DA:P, :])

        def qkv_k_oth(grp):
            pk = ps_tile([DA, 512], F32, "mm2", 2)
            for k in range(KT):
                nc.tensor.matmul(pk, wqkvT[k][:, DA:P],
                                 zhT[:, k, SOWN + grp * 512:SOWN + (grp + 1) * 512],
                                 start=(k == 0), stop=False)
            nc.tensor.matmul(pk, bW[:, DA:P], ones512, start=False, stop=True)
            nc.vector.tensor_copy(out=kTt[:, SOWN + grp * 512:SOWN + (grp + 1) * 512],
                                  in_=pk)

        def qkv_v(grp):
            pv = ps_tile([DA, 512], F32, "mm2", 2)
            for k in range(KT):
                nc.tensor.matmul(pv, wqkvT[k][:, P:3 * DA],
                                 zhT[:, k, grp * 512:(grp + 1) * 512],
                                 start=(k == 0), stop=False)
            nc.tensor.matmul(pv, bW[:, P:3 * DA], ones512, start=False, stop=True)
            nc.vector.tensor_copy(out=vT[:, grp * 512:(grp + 1) * 512], in_=pv)

        for st in range(4):
            do_st(st)
        qkv_qk_own(0)
        for st in range(4, 8):
            do_st(st)
        qkv_qk_own(1)
        qkv_v(0)
        qkv_v(1)
        for st in range(8, 12):
            do_st(st)
        qkv_k_oth(0)
        qkv_v(2)
        for st in range(12, 16):
            do_st(st)
        qkv_k_oth(1)
        qkv_v(3)

        # v natural (+ ones col) via PE transposes: v_aug[h][p_s, jt, 0:32]
        for h in range(NH):
            nc.vector.memset(v_aug[h][:, :, HD:HD + 1], 1.0)
        for jt in range(ST):
            tpv = ps_tile([P, DA], F32, "tpz", 2)
            nc.tensor.matmul(tpv, vT[:, jt * P:(jt + 1) * P], ident[0:DA, 0:DA],
                             start=True, stop=True)
            nc.vector.tensor_copy(out=v_aug[0][:, jt, 0:HD], in_=tpv[:, 0:HD])
            nc.vector.tensor_copy(out=v_aug[1][:, jt, 0:HD], in_=tpv[:, HD:DA])

        # w_base -> WbT via PE transposes; also rowWb[n] = sum_e w_base[n, e]
        WbT = persist.tile([P, KT, DM], BF16, tag="WbT")
        onesCol = persist.tile([P, 1], BF16, tag="onesCol")
        nc.vector.memset(onesCol, 1.0)
        for ntile in range(KT):
            for k in range(KT):
                tpw = ps_tile([P, P], F32, "tpz", 2)
                nc.tensor.matmul(tpw, wbh_t[ntile][:, k * P:(k + 1) * P], ident,
                                 start=True, stop=True)
                if k % 2 == 0:
                    nc.vector.tensor_copy(
                        out=WbT[:, k, ntile * P:(ntile + 1) * P], in_=tpw)
                else:
                    nc.scalar.copy(
                        out=WbT[:, k, ntile * P:(ntile + 1) * P], in_=tpw)
        rowWb = persist.tile([1, DM], BF16, tag="rowWb")
        for g in range(2):
            prw = ps_tile([1, 512], F32, "mm2", 2)
            for k in range(KT):
                nc.tensor.matmul(prw, onesCol, WbT[:, k, g * 512:(g + 1) * 512],
                                 start=(k == 0), stop=(k == KT - 1))
            nc.vector.tensor_copy(out=rowWb[:, g * 512:(g + 1) * 512], in_=prw)

        # ---------------- phase 3: attention (transposed scores) ---------
        def attn_block(h, qg):
            d0 = h * HD
            pao = ps_tile([HD + 1, 512], F32, "mm2", 2)
            psc = ps_tile([P, 2, 512], F32, "sc4", 1)
            for r in range(8):
                for j in range(2):
                    skt = r * 2 + j
                    nc.tensor.matmul(psc[:, j, :],
                                     kTt[d0:d0 + HD, skt * P:(skt + 1) * P],
                                     qT[d0:d0 + HD, qg * 512:(qg + 1) * 512],
                                     start=True, stop=True)
                ext = ex_pool.tile([P, 2, 512], BF16, tag="expt")
                nc.scalar.activation(out=ext, in_=psc, func=AF.Exp,
                                     scale=ATT_SCALE)
                for j in range(2):
                    skt = r * 2 + j
                    nc.tensor.matmul(pao, v_aug[h][:, skt, 0:HD + 1],
                                     ext[:, j, :],
                                     start=(skt == 0), stop=(skt == ST - 1))
            # rr = 1/rowsum (denominator came along as v_aug's ones column)
            rr = st_pool.tile([1, 512], F32, tag="rr")
            nc.vector.reciprocal(out=rr, in_=pao[HD:HD + 1, :])
            rrb = st_pool.tile([HD, 512], F32, tag="rrb")
            nc.gpsimd.partition_broadcast(rrb, rr)
            nc.vector.tensor_tensor(
                out=aoT[d0:d0 + HD, qg * 512:(qg + 1) * 512],
                in0=pao[0:HD, :], in1=rrb,
                op=ALU.mult)

        for h in range(NH):
            for qg in range(2):
                attn_block(h, qg)

        # ---------------- phase 4: attn_out projection + residual --------
        # hwaT = zh*gamma + (waoT.T @ aoT + beta)
        for ntile in range(KT):
            for qg in range(2):
                p4 = ps_tile([P, 512], F32, "mm2", 2)
                nc.tensor.matmul(p4, waoT[:, ntile * P:(ntile + 1) * P],
                                 aoT[:, qg * 512:(qg + 1) * 512],
                                 start=True, stop=False)
                nc.tensor.matmul(p4, bRow[:, ntile * P:(ntile + 1) * P],
                                 ones512, start=False, stop=True)
                nc.vector.scalar_tensor_tensor(
                    out=hwaT[:, ntile, qg * 512:(qg + 1) * 512],
                    in0=zhT[:, ntile, qg * 512:(qg + 1) * 512],
                    scalar=gT[:, ntile:ntile + 1],
                    in1=p4, op0=ALU.mult, op1=ALU.add)

        # ---------------- phase 5: lora down ------------------------------
        for qg in range(2):
            p5 = ps_tile([R, 512], F32, "mm2", 2)
            for k in range(KT):
                nc.tensor.matmul(p5, ld_sb[:, k, :],
                                 hwaT[:, k, qg * 512:(qg + 1) * 512],
                                 start=(k == 0), stop=(k == KT - 1))
            nc.vector.tensor_copy(out=tT[:, qg * 512:(qg + 1) * 512], in_=p5)

        # ---------------- phase 6: base matmul + lora up + output --------
        pot = ps_tile([P, 2, 512], F32, "sc4", 1)
        for mt in range(MT):
            h2 = 0
            for k in range(KT):
                nc.tensor.matmul(pot[:, h2, :], zhT[:, k, mt * P:(mt + 1) * P],
                                 WbT[:, k, 0:512],
                                 start=(k == 0), stop=(k == KT - 1))
                nc.tensor.matmul(pot[:, h2 + 1, :], zhT[:, k, mt * P:(mt + 1) * P],
                                 WbT[:, k, 512:1024],
                                 start=(k == 0), stop=(k == KT - 1))
            rest = [ps_tile([P, 512], F32, "big", 2) for _ in range(2)]
            for g in range(2):
                nc.tensor.matmul(rest[g], muRow[:, mt * P:(mt + 1) * P],
                                 rowWb[:, g * 512:(g + 1) * 512],
                                 start=True, stop=False)
                nc.tensor.matmul(rest[g], tT[:, mt * P:(mt + 1) * P],
                                 lu_sb[:, g * 512:(g + 1) * 512],
                                 start=False, stop=True)
            o_t = o_pool.tile([P, DM], BF16, tag="o_t")
            for g in range(2):
                nc.vector.tensor_scalar_mul(
                    out=o_t[:, g * 512:(g + 1) * 512],
                    in0=pot[:, h2 + g, :], scalar1=stdAll[:, mt:mt + 1])
                nc.vector.tensor_tensor(
                    out=o_t[:, g * 512:(g + 1) * 512],
                    in0=o_t[:, g * 512:(g + 1) * 512], in1=rest[g],
                    op=ALU.add)
            nc.sync.dma_start(out=out_d[mt * P:(mt + 1) * P, :], in_=o_t)

        if dbg:
            for nm, sb in [("zhT", zhT), ("WbT", WbT), ("qT", qT),
                           ("kTt", kTt), ("vT", vT), ("vaug0", v_aug[0]),
                           ("vaug1", v_aug[1]), ("aoT", aoT), ("hwaT", hwaT),
                           ("tT", tT)]:
                nc.sync.dma_start(out=dbg_d[nm], in_=sb)

    nc.compile()
    return nc


_NC_CACHE = None


def _get_nc():
    global _NC_CACHE
    if _NC_CACHE is None:
        _NC_CACHE = build_kernel()
    return _NC_CACHE


def kernel(x, w_base, ln_gamma, ln_beta, lora_down, lora_up, w_qkv, w_attn_out,
           _trace=False):
    x = np.ascontiguousarray(np.asarray(x, dtype=np.float32))
    wk = {
        "w_base": np.ascontiguousarray(np.asarray(w_base, np.float32)),
        "ln_g": np.ascontiguousarray(np.asarray(ln_gamma, np.float32)),
        "ln_b": np.ascontiguousarray(np.asarray(ln_beta, np.float32)),
        "ld": np.ascontiguousarray(np.asarray(lora_down, np.float32)),
        "lu": np.ascontiguousarray(np.asarray(lora_up, np.float32)),
        "w_qkv": np.ascontiguousarray(np.asarray(w_qkv, np.float32)),
        "w_ao": np.ascontiguousarray(np.asarray(w_attn_out, np.float32)),
    }
    nc = _get_nc()
    in_maps = []
    for c in range(NC):
        b, half = divmod(c, 2)
        own = np.ascontiguousarray(x[b, half * SOWN:(half + 1) * SOWN])
        oth = np.ascontiguousarray(x[b, (1 - half) * SOWN:(2 - half) * SOWN])
        in_maps.append({"x_own": own, "x_oth": oth, **wk})
    res = run_bass_kernel_spmd(nc, in_maps, core_ids=list(range(NC)), trace=_trace)
    B, S = x.shape[0], x.shape[1]
    out = np.empty((B, S, DM), np.float32)
    for c in range(NC):
        b, half = divmod(c, 2)
        out[b, half * SOWN:(half + 1) * SOWN] = np.asarray(
            res.results[c]["out"], dtype=np.float32)
    if _trace:
        kernel.last_exec_time_ns = res.exec_time_ns
        kernel.last_results = res
    return out


# revision 16
# speedup vs baseline: 1.0934x; 1.0934x over previous
"""Trainium2 Bass kernel for the LoRA-BC block (nn_LoRABCBlock).

Computation (per reference):
    base = x @ w_base.T
    h = layernorm(x) * gamma + beta
    qkv = h @ w_qkv.T ; attention (2 heads, head_dim 32) over full sequence
    attn_out = attn_output @ w_attn_out.T
    delta = ((h + attn_out) @ lora_down) @ lora_up
    out = base + (1/8) * delta

Sharding: data-parallel over (batch, seq-half) -> 8 cores. Weights
replicated; each core computes k/v over its batch's full 2048 rows.

v3 design notes:
  - PE runs only real matmuls + a few small weight transposes. Big
    transposes (x, z=normalized x, w_base) go through the DMA XBAR
    (dma_start_transpose, bf16). All XBAR transposes are issued on ONE
    HWDGE ring (scalar) so they serialize mutually -- concurrent
    transposes from two rings corrupt each other's xbar state. x loads
    and output stores ride the sync ring so transposes never block them.
  - Attention computes transposed scores scoresT[sk, sq] (lhsT=kT,
    rhs=qT), exp'd in [128, 2048] batches (ACT per-op overhead dominates
    small tiles), and attn@v consumes them directly with a ones column
    appended to V producing the softmax denominator in the same psum.
  - gamma is folded into the qkv weights; beta enters qkv as a rank-1
    matmul, and h(+attn_out) is reconstructed as zh*gamma + (p4 + beta)
    via scalar_tensor_tensor, so no separate gamma/beta pass exists.
  - Output is stored bf16 and widened on the host.
"""

import sys

sys.path.insert(0, "/opt/trn_rl_repo")

from contextlib import ExitStack

import numpy as np

import concourse.bass as bass
import concourse.tile as tile
from concourse import bacc, mybir
from concourse.bass_utils import run_bass_kernel_spmd
from concourse.masks import make_identity

F32 = mybir.dt.float32
BF16 = mybir.dt.bfloat16
AF = mybir.ActivationFunctionType
ALU = mybir.AluOpType

E = 1024          # embed dim
DM = 1024         # d_model
R = 8             # lora rank
SCALING = 1.0 / R
DA = 64           # attn dim
NH = 2            # heads
HD = DA // NH     # head dim = 32
SOWN = 1024       # rows owned per core
SFULL = 2048      # rows per batch element
NC = 8            # cores
P = 128
KT = E // P       # 8 k-tiles
MT = SOWN // P    # 8 own m-tiles
ST = SFULL // P   # 16 sequence tiles
ATT_SCALE = float(HD) ** -0.5


def build_kernel(dbg=False):
    nc = bacc.Bacc("TRN2", target_bir_lowering=False, debug=False, num_devices=NC)

    x_own = nc.dram_tensor("x_own", [SOWN, E], F32, kind="ExternalInput").ap()
    x_oth = nc.dram_tensor("x_oth", [SOWN, E], F32, kind="ExternalInput").ap()
    w_base = nc.dram_tensor("w_base", [DM, E], F32, kind="ExternalInput").ap()
    ln_g = nc.dram_tensor("ln_g", [E], F32, kind="ExternalInput").ap()
    ln_b = nc.dram_tensor("ln_b", [E], F32, kind="ExternalInput").ap()
    ld = nc.dram_tensor("ld", [E, R], F32, kind="ExternalInput").ap()
    lu = nc.dram_tensor("lu", [R, DM], F32, kind="ExternalInput").ap()
    w_qkv = nc.dram_tensor("w_qkv", [3 * DA, E], F32, kind="ExternalInput").ap()
    w_ao = nc.dram_tensor("w_ao", [E, DA], F32, kind="ExternalInput").ap()
    out_d = nc.dram_tensor("out", [SOWN, DM], BF16, kind="ExternalOutput").ap()
    dbg_d = {}
    if dbg:
        for nm, shp in [("zhT", [P, KT, SFULL]),
                        ("WbT", [P, KT, DM]), ("qT", [DA, SOWN]),
                        ("kTt", [DA, SFULL]), ("vT", [DA, SFULL]),
                        ("vaug0", [P, ST, 48]), ("vaug1", [P, ST, 48]),
                        ("aoT", [DA, SOWN]), ("hwaT", [P, KT, SOWN]),
                        ("tT", [R, SOWN])]:
            dbg_d[nm] = nc.dram_tensor("dbg_" + nm, shp, BF16,
                                       kind="ExternalOutput").ap()

    with tile.TileContext(nc) as tc, ExitStack() as ctx:
        persist = ctx.enter_context(tc.tile_pool(name="persist", bufs=1))
        ld_pool = ctx.enter_context(tc.tile_pool(name="loads", bufs=3))
        wb_pool = ctx.enter_context(tc.tile_pool(name="wbload", bufs=2))
        zh_pool = ctx.enter_context(tc.tile_pool(name="zh", bufs=3))
        st_pool = ctx.enter_context(tc.tile_pool(name="stats", bufs=4))
        ex_pool = ctx.enter_context(tc.tile_pool(name="expb", bufs=2))
        o_pool = ctx.enter_context(tc.tile_pool(name="outs", bufs=3))
        ps = ctx.enter_context(tc.tile_pool(name="ps", bufs=1, space="PSUM"))

        _psn = [0]

        def ps_tile(shape, dtype, tag, bufs):
            _psn[0] += 1
            return ps.tile(shape, dtype, tag=tag, bufs=bufs,
                           name=f"ps_{tag}_{_psn[0]}")

        # ---------------- constants ----------------
        ident = persist.tile([P, P], BF16, tag="ident")
        make_identity(nc, ident)
        eps_t = persist.tile([P, 1], F32, tag="eps")
        nc.vector.memset(eps_t, 1e-5)
        ones512 = persist.tile([1, 512], BF16, tag="ones512")
        nc.vector.memset(ones512, 1.0)
        # gamma/beta arranged [p, kt] (e = kt*128 + p)
        gT = persist.tile([P, KT], F32, tag="gT")
        bT = persist.tile([P, KT], F32, tag="bT")
        nc.scalar.dma_start(out=gT, in_=ln_g.rearrange("(kt p) -> p kt", p=P))
        nc.scalar.dma_start(out=bT, in_=ln_b.rearrange("(kt p) -> p kt", p=P))
        bTh = persist.tile([P, KT], BF16, tag="bTh")
        nc.vector.tensor_copy(out=bTh, in_=bT)
        # beta as a bf16 row [1, E] (for the rank-1 +beta in attn residual)
        bRow_f = ld_pool.tile([1, E], F32, tag="bRow_f", bufs=1)
        nc.scalar.dma_start(out=bRow_f, in_=ln_b.rearrange("(o e) -> o e", o=1))
        bRow = persist.tile([1, E], BF16, tag="bRow")
        nc.vector.tensor_copy(out=bRow, in_=bRow_f)

        # ---------------- persistent activations ----------------
        zhT = persist.tile([P, KT, SFULL], BF16, tag="zhT")   # [p_e, k, s] raw z
        qT = persist.tile([DA, SOWN], BF16, tag="qT")
        kTt = persist.tile([DA, SFULL], BF16, tag="kTt")
        vT = persist.tile([DA, SFULL], BF16, tag="vT")
        v_aug = [persist.tile([P, ST, 48], BF16, tag=f"vaug{h}", name=f"vaug{h}")
                 for h in range(NH)]
        aoT = persist.tile([DA, SOWN], BF16, tag="aoT")
        hwaT = persist.tile([P, KT, SOWN], BF16, tag="hwaT")
        tT = persist.tile([R, SOWN], BF16, tag="tT")

        # ---------------- weights (scalar-ring DMAs) ----------------
        # w_qkv -> wqkvT[k] [128e, 192a] via PE transposes; gamma folded in
        wqkvT = [persist.tile([P, 3 * DA], BF16, tag=f"wqkvT{k}", name=f"wqkvT{k}")
                 for k in range(KT)]
        wq0f = ld_pool.tile([P, E], F32, tag="wload", bufs=1)
        nc.scalar.dma_start(out=wq0f, in_=w_qkv[0:P, :])
        wq0h = persist.tile([P, E], BF16, tag="wq0h")
        nc.vector.tensor_copy(out=wq0h, in_=wq0f)
        wq1f = ld_pool.tile([64, E], F32, tag="wload1", bufs=1)
        nc.scalar.dma_start(out=wq1f, in_=w_qkv[P:3 * DA, :])
        wq1h = persist.tile([64, E], BF16, tag="wq1h")
        nc.vector.tensor_copy(out=wq1h, in_=wq1f)
        for k in range(KT):
            tp = ps_tile([P, P], BF16, "mm2", 1)
            nc.tensor.transpose(tp, wq0h[:, k * P:(k + 1) * P], ident)
            nc.vector.tensor_copy(out=wqkvT[k][:, 0:P], in_=tp)
            tp2 = ps_tile([P, 64], BF16, "mm2", 1)
            nc.tensor.transpose(tp2, wq1h[:, k * P:(k + 1) * P], ident[0:64, 0:64])
            nc.vector.tensor_copy(out=wqkvT[k][:, P:3 * DA], in_=tp2)

        # beta @ w_qkv.T  (before gamma folding), then fold gamma into wqkvT
        pbw = ps_tile([1, 3 * DA], F32, "mm2", 1)
        for k in range(KT):
            nc.tensor.matmul(pbw, bTh[:, k:k + 1], wqkvT[k],
                             start=(k == 0), stop=(k == KT - 1))
        bW = persist.tile([1, 3 * DA], BF16, tag="bW")
        nc.vector.tensor_copy(out=bW, in_=pbw)
        for k in range(KT):
            nc.vector.tensor_scalar_mul(out=wqkvT[k], in0=wqkvT[k],
                                        scalar1=gT[:, k:k + 1])

        # w_attn_out -> waoT [64d, 1024n] via PE transposes
        waoT = persist.tile([DA, E], BF16, tag="waoT")
        for ntile in range(KT):
            wf = ld_pool.tile([P, DA], F32, tag="waoload", bufs=2)
            nc.scalar.dma_start(out=wf, in_=w_ao[ntile * P:(ntile + 1) * P, :])
            wh = ld_pool.tile([P, DA], BF16, tag="waocast", bufs=2)
            nc.vector.tensor_copy(out=wh, in_=wf)
            tp = ps_tile([DA, P], BF16, "mm2", 1)
            nc.tensor.transpose(tp, wh, ident)
            nc.vector.tensor_copy(out=waoT[:, ntile * P:(ntile + 1) * P], in_=tp)

        # lora_down [E, R] -> [p, kt, r] bf16
        ld_f = ld_pool.tile([P, KT, R], F32, tag="ldload", bufs=1)
        nc.scalar.dma_start(out=ld_f, in_=ld.rearrange("(kt p) r -> p kt r", p=P))
        ld_sb = persist.tile([P, KT, R], BF16, tag="ld_sb")
        nc.vector.tensor_copy(out=ld_sb, in_=ld_f)

        # lora_up [R, DM] bf16, pre-scaled by SCALING
        lu_f = ld_pool.tile([R, DM], F32, tag="luload", bufs=1)
        nc.scalar.dma_start(out=lu_f, in_=lu)
        lu_sb = persist.tile([R, DM], BF16, tag="lu_sb")
        nc.scalar.mul(lu_sb, lu_f, SCALING)

        # w_base loads + casts (transposes are emitted LAST on the ring)
        wbh_t = []
        for ntile in range(KT):
            wbf = wb_pool.tile([P, E], F32, tag="wbf")
            nc.scalar.dma_start(out=wbf, in_=w_base[ntile * P:(ntile + 1) * P, :])
            wbh = wb_pool.tile([P, E], BF16, tag="wbh", bufs=8)
            nc.vector.tensor_copy(out=wbh, in_=wbf)
            wbh_t.append(wbh)

        stdAll = persist.tile([P, MT], F32, tag="stdAll")   # std per own tile
        muRow = persist.tile([1, SOWN], BF16, tag="muRow")  # mu as row (own)
        stdRow = persist.tile([1, SOWN], F32, tag="stdRow")  # std as row (own)

        # ---------------- phase 1: x load + layernorm -----------------
        def do_st(st):
            own = st < MT
            src = x_own if own else x_oth
            row0 = st * P if own else (st - MT) * P
            xf = ld_pool.tile([P, E], F32, tag="xin")
            nc.sync.dma_start(out=xf, in_=src[row0:row0 + P, :])

            stats = st_pool.tile([P, 2, 6], F32, tag="bnstats")
            xr = xf.rearrange("p (n f) -> p n f", f=512)
            for sg in range(2):
                nc.vector.bn_stats(out=stats[:, sg, :], in_=xr[:, sg, :])
            mv = st_pool.tile([P, 2], F32, tag="mv")
            nc.vector.bn_aggr(out=mv, in_=stats)
            rstd = st_pool.tile([P, 1], F32, tag="rstd")
            nc.scalar.activation(out=rstd, in_=mv[:, 1:2], func=AF.Sqrt, bias=eps_t)
            nc.vector.reciprocal(out=rstd, in_=rstd)
            nmr = st_pool.tile([P, 1], F32, tag="nmr")
            nc.vector.tensor_scalar(out=nmr, in0=mv[:, 0:1], scalar1=rstd,
                                    scalar2=-1.0, op0=ALU.mult, op1=ALU.mult)
            # z = (x - mu) * rstd   (bf16)
            zh = zh_pool.tile([P, E], BF16, tag="zh")
            nc.vector.tensor_scalar(out=zh, in0=xf, scalar1=rstd, scalar2=nmr,
                                    op0=ALU.mult, op1=ALU.add)
            if own:
                # std = 1/rstd and muRow (= mu, via -nmr/rstd... just -nmr*std)
                nc.vector.reciprocal(out=stdAll[:, st:st + 1], in_=rstd)
                # rows: transpose rstd and nmr to [1, 128] rows (partition 0)
                rn = st_pool.tile([P, 2], BF16, tag="rn")
                nc.vector.tensor_copy(out=rn[:, 0:1], in_=rstd)
                nc.vector.tensor_copy(out=rn[:, 1:2], in_=nmr)
                pr0 = ps_tile([1, P], F32, "mm2", 1)
                nc.tensor.matmul(pr0, rn[:, 0:1], ident, start=True, stop=True)
                pr1 = ps_tile([1, P], F32, "mm2", 1)
                nc.tensor.matmul(pr1, rn[:, 1:2], ident, start=True, stop=True)
                nc.vector.reciprocal(out=stdRow[:, st * P:(st + 1) * P],
                                     in_=pr0)
                # mu = -nmr * std  (bf16 row)
                nc.vector.scalar_tensor_tensor(
                    out=muRow[:, st * P:(st + 1) * P], in0=pr1,
                    scalar=-1.0, in1=stdRow[:, st * P:(st + 1) * P],
                    op0=ALU.mult, op1=ALU.mult)
            # transpose z via PE (regular matmuls against identity)
            for k in range(KT):
                tpz = ps_tile([P, P], F32, "tpz", 2)
                nc.tensor.matmul(tpz, zh[:, k * P:(k + 1) * P], ident,
                                 start=True, stop=True)
                if k % 4 != 0:
                    nc.scalar.copy(out=zhT[:, k, st * P:(st + 1) * P], in_=tpz)
                else:
                    nc.vector.tensor_copy(out=zhT[:, k, st * P:(st + 1) * P],
                                          in_=tpz)

        def qkv_qk_own(grp):
            pq = ps_tile([P, 512], F32, "big", 1)
            for k in range(KT):
                nc.tensor.matmul(pq, wqkvT[k][:, 0:P],
                                 zhT[:, k, grp * 512:(grp + 1) * 512],
                                 start=(k == 0), stop=False)
            nc.tensor.matmul(pq, bW[:, 0:P], ones512, start=False, stop=True)
            nc.vector.tensor_copy(out=qT[:, grp * 512:(grp + 1) * 512],
                                  in_=pq[0:DA, :])
            nc.vector.tensor_copy(out=kTt[:, grp * 512:(grp + 1) * 512],
                                  in_=pq[DA:P, :])

        def qkv_k_oth(grp):
            pk = ps_tile([DA, 512], F32, "mm2", 1)
            for k in range(KT):
                nc.tensor.matmul(pk, wqkvT[k][:, DA:P],
                                 zhT[:, k, SOWN + grp * 512:SOWN + (grp + 1) * 512],
                                 start=(k == 0), stop=False)
            nc.tensor.matmul(pk, bW[:, DA:P], ones512, start=False, stop=True)
            nc.vector.tensor_copy(out=kTt[:, SOWN + grp * 512:SOWN + (grp + 1) * 512],
                                  in_=pk)

        def qkv_v(grp):
            pv = ps_tile([DA, 512], F32, "mm2", 1)
            for k in range(KT):
                nc.tensor.matmul(pv, wqkvT[k][:, P:3 * DA],
                                 zhT[:, k, grp * 512:(grp + 1) * 512],
                                 start=(k == 0), stop=False)
            nc.tensor.matmul(pv, bW[:, P:3 * DA], ones512, start=False, stop=True)
            nc.vector.tensor_copy(out=vT[:, grp * 512:(grp + 1) * 512], in_=pv)

        for st in range(4):
            do_st(st)
        qkv_qk_own(0)
        for st in range(4, 8):
            do_st(st)
        qkv_qk_own(1)
        qkv_v(0)
        qkv_v(1)
        for st in range(8, 12):
            do_st(st)
        qkv_k_oth(0)
        qkv_v(2)
        for st in range(12, 16):
            do_st(st)
        qkv_k_oth(1)
        qkv_v(3)

        # v natural (+ ones col) via PE transposes: v_aug[h][p_s, jt, 0:32]
        for h in range(NH):
            nc.vector.memset(v_aug[h][:, :, HD:HD + 1], 1.0)
        for jt in range(ST):
            tpv = ps_tile([P, DA], F32, "tpz", 2)
            nc.tensor.matmul(tpv, vT[:, jt * P:(jt + 1) * P], ident[0:DA, 0:DA],
                             start=True, stop=True)
            nc.vector.tensor_copy(out=v_aug[0][:, jt, 0:HD], in_=tpv[:, 0:HD])
            nc.vector.tensor_copy(out=v_aug[1][:, jt, 0:HD], in_=tpv[:, HD:DA])

        # w_base -> WbT via PE transposes; also rowWb[n] = sum_e w_base[n, e]
        WbT = persist.tile([P, KT, DM], BF16, tag="WbT")
        onesCol = persist.tile([P, 1], BF16, tag="onesCol")
        nc.vector.memset(onesCol, 1.0)
        for ntile in range(KT):
            for k in range(KT):
                tpw = ps_tile([P, P], F32, "tpz", 2)
                nc.tensor.matmul(tpw, wbh_t[ntile][:, k * P:(k + 1) * P], ident,
                                 start=True, stop=True)
                if k % 2 == 0:
                    nc.vector.tensor_copy(
                        out=WbT[:, k, ntile * P:(ntile + 1) * P], in_=tpw)
                else:
                    nc.scalar.copy(
                        out=WbT[:, k, ntile * P:(ntile + 1) * P], in_=tpw)
        rowWb = persist.tile([1, DM], BF16, tag="rowWb")
        for g in range(2):
            prw = ps_tile([1, 512], F32, "mm2", 1)
            for k in range(KT):
                nc.tensor.matmul(prw, onesCol, WbT[:, k, g * 512:(g + 1) * 512],
                                 start=(k == 0), stop=(k == KT - 1))
            nc.vector.tensor_copy(out=rowWb[:, g * 512:(g + 1) * 512], in_=prw)

        # ---------------- phase 3: attention (transposed scores) ---------
        def attn_block(h, qg):
            d0 = h * HD
            pao = ps_tile([HD + 1, 512], F32, "mm2", 1)
            for r in range(8):
                psc = ps_tile([P, 2, 512], F32, "sc4", 2)
                for j in range(2):
                    skt = r * 2 + j
                    nc.tensor.matmul(psc[:, j, :],
                                     kTt[d0:d0 + HD, skt * P:(skt + 1) * P],
                                     qT[d0:d0 + HD, qg * 512:(qg + 1) * 512],
                                     start=True, stop=True)
                ext = ex_pool.tile([P, 2, 512], BF16, tag="expt")
                nc.scalar.activation(out=ext, in_=psc, func=AF.Exp,
                                     scale=ATT_SCALE)
                for j in range(2):
                    skt = r * 2 + j
                    nc.tensor.matmul(pao, v_aug[h][:, skt, 0:HD + 1],
                                     ext[:, j, :],
                                     start=(skt == 0), stop=(skt == ST - 1))
            # rr = 1/rowsum (denominator came along as v_aug's ones column)
            rr = st_pool.tile([1, 512], F32, tag="rr")
            nc.vector.reciprocal(out=rr, in_=pao[HD:HD + 1, :])
            rrb = st_pool.tile([HD, 512], F32, tag="rrb")
            nc.gpsimd.partition_broadcast(rrb, rr)
            nc.vector.tensor_tensor(
                out=aoT[d0:d0 + HD, qg * 512:(qg + 1) * 512],
                in0=pao[0:HD, :], in1=rrb,
                op=ALU.mult)

        for h in range(NH):
            for qg in range(2):
                attn_block(h, qg)

        # ---------------- phase 4: attn_out projection + residual --------
        # hwaT = zh*gamma + (waoT.T @ aoT + beta)
        for ntile in range(KT):
            for qg in range(2):
                p4 = ps_tile([P, 512], F32, "mm2", 1)
                nc.tensor.matmul(p4, waoT[:, ntile * P:(ntile + 1) * P],
                                 aoT[:, qg * 512:(qg + 1) * 512],
                                 start=True, stop=False)
                nc.tensor.matmul(p4, bRow[:, ntile * P:(ntile + 1) * P],
                                 ones512, start=False, stop=True)
                nc.vector.scalar_tensor_tensor(
                    out=hwaT[:, ntile, qg * 512:(qg + 1) * 512],
                    in0=zhT[:, ntile, qg * 512:(qg + 1) * 512],
                    scalar=gT[:, ntile:ntile + 1],
                    in1=p4, op0=ALU.mult, op1=ALU.add)

        # ---------------- phase 5: lora down ------------------------------
        for qg in range(2):
            p5 = ps_tile([R, 512], F32, "mm2", 1)
            for k in range(KT):
                nc.tensor.matmul(p5, ld_sb[:, k, :],
                                 hwaT[:, k, qg * 512:(qg + 1) * 512],
                                 start=(k == 0), stop=(k == KT - 1))
            nc.vector.tensor_copy(out=tT[:, qg * 512:(qg + 1) * 512], in_=p5)

        # ---------------- phase 6: base matmul + lora up + output --------
        for mt in range(MT):
            h2 = 0
            pot = ps_tile([P, 2, 512], F32, "sc4", 2)
            for k in range(KT):
                nc.tensor.matmul(pot[:, h2, :], zhT[:, k, mt * P:(mt + 1) * P],
                                 WbT[:, k, 0:512],
                                 start=(k == 0), stop=(k == KT - 1))
                nc.tensor.matmul(pot[:, h2 + 1, :], zhT[:, k, mt * P:(mt + 1) * P],
                                 WbT[:, k, 512:1024],
                                 start=(k == 0), stop=(k == KT - 1))
            rest = [ps_tile([P, 512], F32, "big", 1),
                    ps_tile([P, 512], F32, "mm2", 1)]
            for g in range(2):
                nc.tensor.matmul(rest[g], muRow[:, mt * P:(mt + 1) * P],
                                 rowWb[:, g * 512:(g + 1) * 512],
                                 start=True, stop=False)
                nc.tensor.matmul(rest[g], tT[:, mt * P:(mt + 1) * P],
                                 lu_sb[:, g * 512:(g + 1) * 512],
                                 start=False, stop=True)
            o_t = o_pool.tile([P, DM], BF16, tag="o_t")
            for g in range(2):
                nc.vector.tensor_scalar_mul(
                    out=o_t[:, g * 512:(g + 1) * 512],
                    in0=pot[:, h2 + g, :], scalar1=stdAll[:, mt:mt + 1])
                nc.vector.tensor_tensor(
                    out=o_t[:, g * 512:(g + 1) * 512],
                    in0=o_t[:, g * 512:(g + 1) * 512], in1=rest[g],
                    op=ALU.add)
            nc.sync.dma_start(out=out_d[mt * P:(mt + 1) * P, :], in_=o_t)

        if dbg:
            for nm, sb in [("zhT", zhT), ("WbT", WbT), ("qT", qT),
                           ("kTt", kTt), ("vT", vT), ("vaug0", v_aug[0]),
                           ("vaug1", v_aug[1]), ("aoT", aoT), ("hwaT", hwaT),
                           ("tT", tT)]:
                nc.sync.dma_start(out=dbg_d[nm], in_=sb)

    nc.compile()
    return nc


_NC_CACHE = None


def _get_nc():
    global _NC_CACHE
    if _NC_CACHE is None:
        _NC_CACHE = build_kernel()
    return _NC_CACHE


def kernel(x, w_base, ln_gamma, ln_beta, lora_down, lora_up, w_qkv, w_attn_out,
           _trace=False):
    x = np.ascontiguousarray(np.asarray(x, dtype=np.float32))
    wk = {
        "w_base": np.ascontiguousarray(np.asarray(w_base, np.float32)),
        "ln_g": np.ascontiguousarray(np.asarray(ln_gamma, np.float32)),
        "ln_b": np.ascontiguousarray(np.asarray(ln_beta, np.float32)),
        "ld": np.ascontiguousarray(np.asarray(lora_down, np.float32)),
        "lu": np.ascontiguousarray(np.asarray(lora_up, np.float32)),
        "w_qkv": np.ascontiguousarray(np.asarray(w_qkv, np.float32)),
        "w_ao": np.ascontiguousarray(np.asarray(w_attn_out, np.float32)),
    }
    nc = _get_nc()
    in_maps = []
    for c in range(NC):
        b, half = divmod(c, 2)
        own = np.ascontiguousarray(x[b, half * SOWN:(half + 1) * SOWN])
        oth = np.ascontiguousarray(x[b, (1 - half) * SOWN:(2 - half) * SOWN])
        in_maps.append({"x_own": own, "x_oth": oth, **wk})
    res = run_bass_kernel_spmd(nc, in_maps, core_ids=list(range(NC)), trace=_trace)
    B, S = x.shape[0], x.shape[1]
    out = np.empty((B, S, DM), np.float32)
    for c in range(NC):
        b, half = divmod(c, 2)
        out[b, half * SOWN:(half + 1) * SOWN] = np.asarray(
            res.results[c]["out"], dtype=np.float32)
    if _trace:
        kernel.last_exec_time_ns = res.exec_time_ns
        kernel.last_results = res
    return out
